# revision 75
# baseline (speedup 1.0000x reference)
"""GroupedAttention Trainium2 kernel.

Problem: x[2048, 2, 256]; K/V projections to G=2 groups (head width 256),
Q projection to G*SUB=8 heads; LayerNorm on K and Q; causal softmax
attention per (b, g, sub); output projection back to 256.

Sharding: 16 (b, g, sub) heads over 8 cores -> 2 heads per core.
Core c: b = c//4, g = (c//2)%2, sub-pair j = c%2 (subs 2j, 2j+1).
The host sums the 4 partials per batch and adds a folded constant bias
(WO_b plus every head's V-bias contribution through WO).

Key structure (tuned against the TRN2 cost-model timeline):
- K^T and Q^T are produced DIRECTLY by matmuls (weights stationary, x^T
  moving), eliminating every K/Q transpose on the PE. x and the
  projection weights travel as bf16 (all matmuls accumulate in fp32
  PSUM); the first weight/x block is packed into one fused header DMA.
- LayerNorm is never applied to K. Scores use raw (biased) K^T; the
  per-key factor 1/(16*std_k) folds into the Exp activation's
  per-partition scale AP, and the mean term cancels because the
  normalized Q rows sum to ~0 (requires ln_g == 1, ln_b == 0, which
  kernel() asserts). Per-key mean/sumsq come from N=2 matmuls against
  duplicated row-mean weight columns and a Square + ones-column
  reduction, batched 4 seq-tiles per PSUM bank (fp32r ISA rules: even
  moving counts, outputs at partition 0).
- Q IS normalized (its per-query scale sits inside the softmax):
  mean/sumsq rows come from M=1 matmuls into partition-0 PSUM rows,
  are converted to (mu, 1/std) rows, broadcast across partitions on
  the otherwise-idle GPSIMD engine, and applied with one
  scalar_tensor_tensor (bias add + mean subtract) plus one multiply.
- Causal structure at 128-tile granularity: score columns below the
  diagonal tile are skipped entirely (bf16 moving keeps 1 cyc/row even
  below 256 columns), PV matmuls for empty tile pairs are skipped, and
  only the diagonal 128x128 tile is masked (one shared 0/1 bf16 mask).
- A ones-column appended to V makes PSUM column 256 accumulate the
  softmax denominator for free; the reciprocal is applied per query
  tile as soon as that tile's accumulation stops.
- The kt loop is software-pipelined (scores issued three blocks ahead
  of PV); O-transposes are batched 4-per-bank with one wide PSUM->SBUF
  copy, and the previous superblock's output projection plus the other
  head's transposes are injected into the next superblock's stream so
  the PE never waits on the DVE chains that feed them. The final
  superblock drains at per-query-tile granularity.
- PSUM (8 banks, 1 bank per tile slot) is phase-scoped: projection
  pools (proj x3, K-stats x1, Q-rows x4) close before the attention
  pools (scores x2, O-accumulators x4, transpose/output x2) open.
- bf16 for K^T/Q^T/P/V/O tiles, WO, and the transpose identity
  (transposes run 1.0 cyc/row vs 1.5 for f32r).
- A dummy Sqrt primes the sqrt-capable activation table before the
  first Square so the scalar engine loads its table once per phase.
"""

import sys

import numpy as np

for _p in ("/opt/trn_rl_repo",):
    if _p not in sys.path:
        sys.path.insert(0, _p)

SEQ, BS, DIM = 2048, 2, 256
G, SUB = 2, 4
N_CORES = 8
LN_EPS = 1e-5
NT = SEQ // 128  # 16 seq tiles of 128
NSB = SEQ // 512  # 4 blocks of 512 (query superblocks / proj blocks)

_CACHE = {}


def _build_program():
    from contextlib import ExitStack

    import concourse.bacc as bacc
    import concourse.bass_isa as bass_isa
    import concourse.mybir as mybir
    from concourse import tile
    f32 = mybir.dt.float32
    f32r = mybir.dt.float32r
    bf16 = mybir.dt.bfloat16
    AF = mybir.ActivationFunctionType
    OP = mybir.AluOpType

    nc = bacc.Bacc("TRN2", target_bir_lowering=False, debug=False)

    xt_d = nc.dram_tensor("xt", [128, 2, SEQ], bf16, kind="ExternalInput").ap()
    hdr_d = nc.dram_tensor("hdr", [128, 2560], bf16, kind="ExternalInput").ap()
    bkc_d = nc.dram_tensor("bkc", [128, 2], f32, kind="ExternalInput").ap()
    wmk_d = nc.dram_tensor("wmk", [128, 4], bf16, kind="ExternalInput").ap()
    bkm_d = nc.dram_tensor("bkm", [128, 1], f32, kind="ExternalInput").ap()
    bqc_d = nc.dram_tensor("bqc", [128, 4], f32, kind="ExternalInput").ap()
    wmq_d = nc.dram_tensor("wmq", [128, 2, 2], bf16, kind="ExternalInput").ap()
    bqm_d = nc.dram_tensor("bqm", [1, 2], f32, kind="ExternalInput").ap()
    wv_d = nc.dram_tensor("wv", [128, 2, 256], bf16, kind="ExternalInput").ap()
    wo_d = nc.dram_tensor("wo", [128, 4, 256], bf16, kind="ExternalInput").ap()
    id_d = nc.dram_tensor("ident", [128, 128], bf16, kind="ExternalInput").ap()
    cm_d = nc.dram_tensor("cmask", [128, 128], bf16, kind="ExternalInput").ap()
    vo_d = nc.dram_tensor("vpones", [128, NT, 2], bf16, kind="ExternalInput").ap()
    out_d = nc.dram_tensor("out_partial", [SEQ, DIM], f32, kind="ExternalOutput").ap()

    r = lambda ap: ap.bitcast(f32r)

    with tile.TileContext(nc) as tc, ExitStack() as ctx:
        const = ctx.enter_context(tc.tile_pool(name="const", bufs=1))

        xt_sb = const.tile([128, 2, SEQ], bf16)
        hdr_sb = const.tile([128, 2560], bf16)
        bkc_sb = const.tile([128, 2], f32)
        wmk_sb = const.tile([128, 4], bf16)
        bkm_sb = const.tile([128, 1], f32)
        bqc_sb = const.tile([128, 4], f32)
        wmq_sb = const.tile([128, 2, 2], bf16)
        bqm_sb = const.tile([1, 2], f32)
        wv_sb = const.tile([128, 2, 256], bf16)
        wo_sb = const.tile([128, 4, 256], bf16)
        ident_sb = const.tile([128, 128], bf16)
        tmask_sb = const.tile([128, 128], bf16)
        onescol_sb = const.tile([128, 2], f32)
        epsk_sb = const.tile([128, 1], f32)
        epsq_sb = const.tile([1, 1], f32)

        # persistent SBUF activations
        ktb = [
            [const.tile([128, 512], bf16, name=f"ktb{f}_{b}") for b in range(NSB)]
            for f in range(2)
        ]
        qtn = [
            [const.tile([128, 512], bf16, name=f"qtn{fc}_{b}") for b in range(NSB)]
            for fc in range(4)
        ]
        vp_t = [const.tile([128, 258], bf16, name=f"vpt{t}") for t in range(NT)]
        rk16b = [const.tile([128, 8], f32, name=f"rk16b{b}") for b in range(NSB)]
        otb = [
            [const.tile([128, 512], bf16, name=f"otb{c}_{s}") for s in range(NSB)]
            for c in range(4)
        ]

        nc.gpsimd.memset(onescol_sb[:], 1.0)
        nc.gpsimd.memset(epsk_sb[:], 256.0 * LN_EPS)
        nc.gpsimd.memset(epsq_sb[:], LN_EPS)
        # prime the sqrt-capable activation table before any Square lands
        warm_sb = const.tile([1, 1], f32)
        nc.scalar.activation(warm_sb[:], epsq_sb[:], AF.Sqrt)
        nc.sync.dma_start(hdr_sb[:, 0:1024], hdr_d[:, 0:1024])
        nc.sync.dma_start(hdr_sb[:, 1024:1536], hdr_d[:, 1024:1536])
        nc.sync.dma_start(hdr_sb[:, 1536:2560], hdr_d[:, 1536:2560])
        nc.sync.dma_start(bkc_sb[:], bkc_d[:])
        nc.sync.dma_start(wmk_sb[:], wmk_d[:])
        nc.sync.dma_start(bkm_sb[:], bkm_d[:])
        nc.sync.dma_start(bqc_sb[:], bqc_d[:])
        nc.sync.dma_start(wmq_sb[:], wmq_d[:])
        nc.sync.dma_start(bqm_sb[:], bqm_d[:])
        nc.sync.dma_start(wv_sb[:], wv_d[:])
        nc.sync.dma_start(xt_sb[:, :, 512:1024], xt_d[:, :, 512:1024])
        nc.sync.dma_start(xt_sb[:, :, 1024:1536], xt_d[:, :, 1024:1536])
        nc.sync.dma_start(xt_sb[:, :, 1536:2048], xt_d[:, :, 1536:2048])
        nc.sync.dma_start(ident_sb[:], id_d[:])
        nc.sync.dma_start(tmask_sb[:], cm_d[:])
        for t in range(NT):
            nc.sync.dma_start(vp_t[t][:, 256:258], vo_d[:, t, :])
        nc.sync.dma_start(wo_sb[:], wo_d[:])


        def wk_v(c, f):
            return hdr_sb[:, c * 256 + f * 128 : c * 256 + (f + 1) * 128]

        def wq_v(c, lo, hi):
            return hdr_sb[:, 1536 + c * 512 + lo : 1536 + c * 512 + hi]

        def xt_v(c, lo, hi):
            if hi <= 512:
                base = 512 + c * 512
                return hdr_sb[:, base + lo : base + hi]
            return xt_sb[:, c, lo:hi]

        wrk = ctx.enter_context(tc.tile_pool(name="wrk", bufs=3))
        ppool = ctx.enter_context(tc.tile_pool(name="ppool", bufs=6))
        opool = ctx.enter_context(tc.tile_pool(name="opool", bufs=6))

        # ======== projection phase (pools scoped; close before attention) ====
        with tc.tile_pool(name="projp", bufs=3, space="PSUM") as projp, \
             tc.tile_pool(name="tinyp", bufs=1, space="PSUM") as tinyp, \
             tc.tile_pool(name="rowp", bufs=1, space="PSUM") as rowp:
            for B in range(NSB):
                sl = slice(B * 512, (B + 1) * 512)
                # --- K^T chunks: biased SBUF copy + biased square (from PSUM)
                ktsq = []
                for f in range(2):
                    kps = projp.tile([128, 512], f32, tag="proj", name=f"kps{f}_{B}")
                    for c in range(2):
                        nc.tensor.matmul(
                            kps[:],
                            lhsT=wk_v(c, f),
                            rhs=xt_v(c, B * 512, (B + 1) * 512),
                            start=(c == 0),
                            stop=(c == 1),
                        )
                    ksq = wrk.tile([128, 512], f32r, tag=f"ksq{f}", bufs=2,
                                   name=f"ksq{f}_{B}")
                    nc.scalar.activation(
                        ksq[:], kps[:], AF.Square, bias=bkc_sb[:, f : f + 1]
                    )
                    nc.vector.tensor_scalar_add(
                        ktb[f][B][:], kps[:], scalar1=bkc_sb[:, f : f + 1]
                    )
                    ktsq.append(ksq)
                # --- Q^T chunks: raw in PSUM until normalize; biased square
                qps_l, qtsq = [], []
                for fc in range(4):
                    qps = projp.tile([128, 512], f32, tag="proj", name=f"qps{fc}_{B}")
                    for c in range(2):
                        nc.tensor.matmul(
                            qps[:],
                            lhsT=wq_v(c, fc * 128, (fc + 1) * 128),
                            rhs=xt_v(c, B * 512, (B + 1) * 512),
                            start=(c == 0),
                            stop=(c == 1),
                        )
                    qsq = wrk.tile([128, 512], f32r, tag=f"qsq{fc}", bufs=2,
                                   name=f"qsq{fc}_{B}")
                    nc.scalar.activation(
                        qsq[:], qps[:], AF.Square, bias=bqc_sb[:, fc : fc + 1]
                    )
                    qps_l.append(qps)
                    qtsq.append(qsq)
                # --- V tiles ---
                for t in range(4 * B, 4 * B + 4):
                    vps = projp.tile([128, 256], f32, tag="proj", name=f"vps{t}")
                    for c in range(2):
                        nc.tensor.matmul(
                            vps[:],
                            lhsT=xt_v(c, t * 128, (t + 1) * 128),
                            rhs=wv_sb[:, c, :],
                            start=(c == 0),
                            stop=(c == 1),
                        )
                    nc.scalar.copy(vp_t[t][:, 0:256], vps[:])
                # --- K per-key stats: mu (cols 0:4) and sumsq (cols 4:8) ---
                tiny = tinyp.tile([128, 16], f32, tag="tiny", name=f"tiny{B}")
                for i, t in enumerate(range(4 * B, 4 * B + 4)):
                    for c in range(2):
                        nc.tensor.matmul(
                            tiny[:, 2 * i : 2 * i + 2],
                            lhsT=xt_v(c, t * 128, (t + 1) * 128),
                            rhs=wmk_sb[:, 2 * c : 2 * c + 2],
                            start=(c == 0),
                            stop=(c == 1),
                        )
                    for f in range(2):
                        nc.tensor.matmul(
                            tiny[:, 8 + 2 * i : 10 + 2 * i],
                            lhsT=r(ktsq[f][:, i * 128 : (i + 1) * 128]),
                            rhs=r(onescol_sb[:, 0:2]),
                            start=(f == 0),
                            stop=(f == 1),
                        )
                # --- Q row stats, one bank per head: mu at partition 0,
                # sumsq at partition 32 (matmul base must be 0/32/64)
                mur_h = [
                    rowp.tile([1, 512], f32, tag=f"mur{h}", name=f"mur{h}_{B}")[:]
                    for h in range(2)
                ]
                for h in range(2):
                    for c in range(2):
                        nc.tensor.matmul(
                            mur_h[h],
                            lhsT=wmq_sb[:, c, h : h + 1],
                            rhs=xt_v(c, B * 512, (B + 1) * 512),
                            start=(c == 0),
                            stop=(c == 1),
                        )
                sqr_h = [
                    rowp.tile([1, 512], f32, tag=f"sqr{h}", name=f"sqr{h}_{B}")[:]
                    for h in range(2)
                ]
                for h in range(2):
                    for c in range(2):
                        nc.tensor.matmul(
                            sqr_h[h],
                            lhsT=r(onescol_sb[:, 0:1]),
                            rhs=r(qtsq[2 * h + c][:]),
                            start=(c == 0),
                            stop=(c == 1),
                        )
                # --- K stats -> rk16 (batched over the 4 seq tiles) ---
                mu2 = wrk.tile([128, 8], f32, tag="mu2", name=f"mu2_{B}")
                nc.scalar.activation(mu2[:], tiny[:, 0:8], AF.Square, bias=bkm_sb[:])
                v256 = wrk.tile([128, 8], f32, tag="v256", name=f"v256_{B}")
                nc.vector.scalar_tensor_tensor(
                    v256[:], mu2[:], -256.0, tiny[:, 8:16],
                    op0=OP.mult, op1=OP.add,
                )
                std16 = wrk.tile([128, 8], f32, tag="std16", name=f"std16_{B}")
                nc.scalar.activation(std16[:], v256[:], AF.Sqrt, bias=epsk_sb[:])
                nc.vector.reciprocal(rk16b[B][:], std16[:])
                # --- Q row stats -> (mu_biased, 1/std) rows + broadcasts ---
                mub = [None, None]
                rqb = [None, None]
                for h in range(2):
                    murow = wrk.tile([1, 512], f32, tag=f"murow{h}", bufs=1,
                                     name=f"murow{h}_{B}")
                    nc.vector.tensor_scalar_add(
                        murow[:], mur_h[h],
                        scalar1=bqm_sb[0:1, h : h + 1],
                    )
                    mu2r = wrk.tile([1, 512], f32, tag=f"mu2r{h}", bufs=1,
                                    name=f"mu2r{h}_{B}")
                    nc.scalar.activation(mu2r[:], murow[:], AF.Square)
                    v256r = wrk.tile([1, 512], f32, tag=f"v256r{h}", bufs=1,
                                     name=f"v256r{h}_{B}")
                    nc.vector.scalar_tensor_tensor(
                        v256r[:], mu2r[:], -256.0, sqr_h[h],
                        op0=OP.mult, op1=OP.add,
                    )
                    stdr = wrk.tile([1, 512], f32, tag=f"stdr{h}", bufs=1,
                                    name=f"stdr{h}_{B}")
                    nc.scalar.activation(
                        stdr[:], v256r[:], AF.Sqrt, bias=epsq_sb[:],
                        scale=1.0 / 256.0,
                    )
                    rqrow = wrk.tile([1, 512], f32, tag=f"rqrow{h}", bufs=1,
                                     name=f"rqrow{h}_{B}")
                    nc.vector.reciprocal(rqrow[:], stdr[:])
                    mub[h] = wrk.tile([128, 512], f32, tag=f"mub{h}", bufs=1,
                                      name=f"mub{h}_{B}")
                    nc.gpsimd.partition_broadcast(mub[h][:], murow[:])
                    rqb[h] = wrk.tile([128, 512], f32, tag=f"rqb{h}", bufs=1,
                                      name=f"rqb{h}_{B}")
                    nc.gpsimd.partition_broadcast(rqb[h][:], rqrow[:])
                # --- normalize Q: ((raw + bias) - mu) * (1/std) -> bf16 ---
                for fc in range(4):
                    h = fc // 2
                    qtmp = wrk.tile([128, 512], f32, tag=f"qtmp{fc % 2}",
                                    name=f"qtmp{fc}_{B}")
                    nc.vector.scalar_tensor_tensor(
                        qtmp[:], qps_l[fc][:], bqc_sb[:, fc : fc + 1], mub[h][:],
                        op0=OP.add, op1=OP.subtract,
                    )
                    nc.gpsimd.tensor_mul(qtn[fc][B][:], qtmp[:], rqb[h][:])

        # ======== attention phase ========
        psA = ctx.enter_context(tc.tile_pool(name="psA", bufs=2, space="PSUM"))
        psB = ctx.enter_context(tc.tile_pool(name="psB", bufs=1, space="PSUM"))
        psT = ctx.enter_context(tc.tile_pool(name="psT", bufs=2, space="PSUM"))

        def attn_superblock(h, s, inject=None):
            n_k = 4 * (s + 1)
            oacc = [
                psB.tile([128, 258], f32, tag=f"oacc{j}", name=f"oacc{h}_{s}_{j}")
                for j in range(4)
            ]
            p_tiles = [None] * n_k

            def issue_scores(kt):
                d = kt - 4 * s  # >= 0 on the diagonal region
                qoff = 0 if d <= 0 else d * 128  # bf16 moving: 1 cyc/row anyway
                st = psA.tile([128, 512], f32, tag="mm512", name=f"st{h}_{s}_{kt}")
                for c in range(2):
                    nc.tensor.matmul(
                        st[:, qoff:512],
                        lhsT=ktb[c][kt // 4][:, (kt % 4) * 128 : (kt % 4 + 1) * 128],
                        rhs=qtn[h * 2 + c][s][:, qoff:512],
                        start=(c == 0),
                        stop=(c == 1),
                    )
                p = ppool.tile([128, 512], bf16, tag="p", name=f"p{h}_{s}_{kt}")
                nc.scalar.activation(
                    p[:, qoff:512], st[:, qoff:512], AF.Exp,
                    scale=rk16b[kt // 4][:, 2 * (kt % 4) : 2 * (kt % 4) + 1],
                )
                if d >= 0:
                    nc.vector.tensor_mul(
                        p[:, d * 128 : (d + 1) * 128],
                        p[:, d * 128 : (d + 1) * 128],
                        tmask_sb[:],
                    )
                p_tiles[kt] = p

            def issue_pv(kt):
                d = kt - 4 * s
                p = p_tiles[kt]
                for j in range(max(d, 0), 4):
                    nc.tensor.matmul(
                        oacc[j][:],
                        lhsT=p[:, j * 128 : (j + 1) * 128],
                        rhs=vp_t[kt][:],
                        start=(kt == 0),
                        stop=(kt == 4 * s + j),
                    )

            for kk in range(min(3, n_k)):
                issue_scores(kk)
            if inject is not None:
                inject()
            osb_list = [None] * 4

            def finish_j(j):
                rc = wrk.tile([128, 1], f32, tag="rc", name=f"rc{h}_{s}_{j}")
                nc.vector.reciprocal(rc[:], oacc[j][:, 256:257])
                osb = opool.tile([128, 256], bf16, tag="osb", name=f"osb{h}_{s}_{j}")
                nc.vector.tensor_scalar_mul(osb[:], oacc[j][:, 0:256], rc[:])
                osb_list[j] = osb

            for kt in range(n_k):
                issue_pv(kt)
                if kt + 3 < n_k:
                    issue_scores(kt + 3)
                if kt >= 4 * s:
                    finish_j(kt - 4 * s)
            return osb_list

        def o_trans(h, s, osb_list):
            """transpose the 4 normalized output tiles of (h, s) into otb"""
            for c in range(2):
                big = psT.tile([128, 512], bf16, tag="tp", name=f"obig{h}_{s}_{c}")
                for j in range(4):
                    nc.tensor.matmul(
                        big[:, j * 128 : (j + 1) * 128],
                        lhsT=osb_list[j][:, c * 128 : (c + 1) * 128],
                        rhs=ident_sb[:],
                        is_transpose=True,
                    )
                nc.vector.tensor_copy(otb[h * 2 + c][s][:], big[:])

        def o_proj(t):
            s, j = t // 4, t % 4
            ops = psT.tile([128, 256], f32, tag="tp", name=f"ops{t}")
            for c in range(4):
                nc.tensor.matmul(
                    ops[:],
                    lhsT=otb[c][s][:, j * 128 : (j + 1) * 128],
                    rhs=wo_sb[:, c, :],
                    start=(c == 0),
                    stop=(c == 3),
                )
            outsb = opool.tile([128, 256], f32, tag="outsb", name=f"outsb{t}")
            nc.scalar.copy(outsb[:], ops[:])
            nc.sync.dma_start(out_d[t * 128 : (t + 1) * 128, :], outsb[:])

        # pipeline: h0(s) transposes + h1(s-1) transposes + o_proj(s-1) all
        # run inside later instruction streams so their DVE inputs are ready.
        osb_mem = {}
        for s in range(NSB):

            def inj_h0(ss=s):
                if ss > 0:
                    o_trans(1, ss - 1, osb_mem[(1, ss - 1)])

            def inj_h1(ss=s):
                if ss > 0:
                    for t in range(4 * (ss - 1), 4 * ss):
                        o_proj(t)
                o_trans(0, ss, osb_mem[(0, ss)])

            osb_mem[(0, s)] = attn_superblock(0, s, inj_h0)
            osb_mem[(1, s)] = attn_superblock(1, s, inj_h1)

        # drain tail, j-granular: transpose/copy/project/store per query tile
        s_last = NSB - 1
        osb_l = osb_mem[(1, s_last)]

        def tail_trans(j):
            jb = psT.tile([128, 256], bf16, tag="tp", name=f"jb{j}")
            for c in range(2):
                nc.tensor.matmul(
                    jb[:, c * 128 : (c + 1) * 128],
                    lhsT=osb_l[j][:, c * 128 : (c + 1) * 128],
                    rhs=ident_sb[:],
                    is_transpose=True,
                )
            for c in range(2):
                nc.vector.tensor_copy(
                    otb[2 + c][s_last][:, j * 128 : (j + 1) * 128],
                    jb[:, c * 128 : (c + 1) * 128],
                )

        for j in range(4):
            tail_trans(j)
            o_proj(4 * s_last + j)

    nc.finalize()
    return nc


def _chunk2(a):
    """[256, F] -> [128, 2, F] (feature chunks on the free axis)."""
    f = a.shape[1]
    return np.ascontiguousarray(a.reshape(2, 128, f).transpose(1, 0, 2))


def _prep_core_inputs(c, x, WK_w, WK_b, WV_w, WV_b, WQ_w, WQ_b, WO_w):
    import ml_dtypes

    bf16 = ml_dtypes.bfloat16
    b, g, j = c // 4, (c // 2) % 2, c % 2
    f32 = np.float32

    xT = np.ascontiguousarray(x[:, b, :].T.astype(f32))  # [256, 2048]
    xt = _chunk2(xT).astype(bf16)

    wk_s = WK_w[:, g * 256 : (g + 1) * 256].astype(f32)  # [256 in, 256 out]
    bk_s = WK_b[g * 256 : (g + 1) * 256].astype(f32)
    wv_s = np.ascontiguousarray(WV_w[:, g * 256 : (g + 1) * 256].astype(f32))

    col = (g * SUB + 2 * j) * 256
    wq_s = WQ_w[:, col : col + 512].astype(f32)  # both heads [256 in, 512 out]
    bq_s = WQ_b[col : col + 512].astype(f32)

    row = (g * SUB + 2 * j) * 256
    wo_s = WO_w[row : row + 512, :].astype(f32)  # [512, 256]
    wo = np.ascontiguousarray(wo_s.reshape(4, 128, 256).transpose(1, 0, 2))

    pp, ff = np.meshgrid(np.arange(128), np.arange(128), indexing="ij")
    hdr = np.concatenate(
        [
            np.ascontiguousarray(_chunk2(wk_s).reshape(128, 512)).astype(bf16),
            xt[:, 0, 0:512],
            xt[:, 1, 0:512],
            np.ascontiguousarray(_chunk2(wq_s).reshape(128, 1024)).astype(bf16),
        ],
        axis=1,
    )
    return {
        "xt": xt,
        "hdr": np.ascontiguousarray(hdr),
        "bkc": np.ascontiguousarray(bk_s.reshape(2, 128).T),
        "wmk": np.ascontiguousarray(
            np.repeat(wk_s.mean(axis=1, keepdims=True).reshape(2, 128).T, 2, axis=1)
        ).astype(bf16),
        "bkm": np.full((128, 1), bk_s.mean(), dtype=f32),
        "bqc": np.ascontiguousarray(bq_s.reshape(4, 128).T),
        "wmq": np.ascontiguousarray(
            wq_s.reshape(256, 2, 256).mean(axis=2).reshape(2, 128, 2).transpose(1, 0, 2)
        ).astype(bf16),
        "bqm": np.array(
            [[bq_s[0:256].mean(), bq_s[256:512].mean()]], dtype=f32
        ),
        "wv": _chunk2(wv_s).astype(bf16),
        "wo": wo.astype(bf16),
        "ident": np.eye(128, dtype=bf16),
        "cmask": (pp <= ff).astype(bf16),  # keep k<=q on the diagonal tile
        "vpones": np.concatenate(
            [np.ones((128, NT, 1), dtype=bf16), np.zeros((128, NT, 1), dtype=bf16)],
            axis=2,
        ),
    }


def kernel(x, WK_w, WK_b, WV_w, WV_b, WQ_w, WQ_b, WO_w, WO_b, ln_g, ln_b, **kwargs):
    x = np.asarray(x)
    WK_w, WK_b = np.asarray(WK_w), np.asarray(WK_b)
    WV_w, WV_b = np.asarray(WV_w), np.asarray(WV_b)
    WQ_w, WQ_b = np.asarray(WQ_w), np.asarray(WQ_b)
    WO_w, WO_b = np.asarray(WO_w), np.asarray(WO_b)
    ln_g, ln_b = np.asarray(ln_g), np.asarray(ln_b)

    if not np.allclose(ln_b, 0.0):
        raise NotImplementedError("nonzero ln_b not supported by this kernel")
    if not np.allclose(ln_g, 1.0):
        raise NotImplementedError("non-unit ln_g not supported by this kernel")

    if "nc" not in _CACHE:
        _CACHE["nc"] = _build_program()
    nc = _CACHE["nc"]

    in_maps = [
        _prep_core_inputs(c, x, WK_w, WK_b, WV_w, WV_b, WQ_w, WQ_b, WO_w)
        for c in range(N_CORES)
    ]

    from concourse.bass_utils import run_bass_kernel_spmd

    res = run_bass_kernel_spmd(nc, in_maps, list(range(N_CORES)))
    results = res.results

    out = np.zeros((SEQ, BS, DIM), dtype=np.float32)
    for c in range(N_CORES):
        out[:, c // 4, :] += results[c]["out_partial"]

    # fold: WO_b plus the V-bias contribution of every head
    const_bias = WO_b.astype(np.float64).copy()
    for g in range(G):
        bv = WV_b[g * 256 : (g + 1) * 256].astype(np.float64)
        for sh in range(SUB):
            row = (g * SUB + sh) * 256
            const_bias += bv @ WO_w[row : row + 256, :].astype(np.float64)
    out += const_bias.astype(np.float32)[None, None, :]
    return out


# revision 78
# speedup vs baseline: 1.0015x; 1.0015x over previous
"""GroupedAttention Trainium2 kernel.

Problem: x[2048, 2, 256]; K/V projections to G=2 groups (head width 256),
Q projection to G*SUB=8 heads; LayerNorm on K and Q; causal softmax
attention per (b, g, sub); output projection back to 256.

Sharding: 16 (b, g, sub) heads over 8 cores -> 2 heads per core.
Core c: b = c//4, g = (c//2)%2, sub-pair j = c%2 (subs 2j, 2j+1).
The host sums the 4 partials per batch and adds a folded constant bias
(WO_b plus every head's V-bias contribution through WO).

Key structure (tuned against the TRN2 cost-model timeline):
- K^T and Q^T are produced DIRECTLY by matmuls (weights stationary, x^T
  moving), eliminating every K/Q transpose on the PE. x and the
  projection weights travel as bf16 (all matmuls accumulate in fp32
  PSUM); the first weight/x block is packed into one fused header DMA.
- LayerNorm is never applied to K. Scores use raw (biased) K^T; the
  per-key factor 1/(16*std_k) folds into the Exp activation's
  per-partition scale AP, and the mean term cancels because the
  normalized Q rows sum to ~0 (requires ln_g == 1, ln_b == 0, which
  kernel() asserts). Per-key mean/sumsq come from N=2 matmuls against
  duplicated row-mean weight columns and a Square + ones-column
  reduction, batched 4 seq-tiles per PSUM bank (fp32r ISA rules: even
  moving counts, outputs at partition 0).
- Q IS normalized (its per-query scale sits inside the softmax):
  mean/sumsq rows come from M=1 matmuls into partition-0 PSUM rows,
  are converted to (mu, 1/std) rows, broadcast across partitions on
  the otherwise-idle GPSIMD engine, and applied with one
  scalar_tensor_tensor (bias add + mean subtract) plus one multiply.
- Causal structure at 128-tile granularity: score columns below the
  diagonal tile are skipped entirely (bf16 moving keeps 1 cyc/row even
  below 256 columns), PV matmuls for empty tile pairs are skipped, and
  only the diagonal 128x128 tile is masked (one shared 0/1 bf16 mask).
- A ones-column appended to V makes PSUM column 256 accumulate the
  softmax denominator for free; the reciprocal is applied per query
  tile as soon as that tile's accumulation stops.
- The kt loop is software-pipelined (scores issued three blocks ahead
  of PV); O-transposes are batched 4-per-bank with one wide PSUM->SBUF
  copy, and the previous superblock's output projection plus the other
  head's transposes are injected into the next superblock's stream so
  the PE never waits on the DVE chains that feed them. The final
  superblock drains at per-query-tile granularity.
- PSUM (8 banks, 1 bank per tile slot) is phase-scoped: projection
  pools (proj x3, K-stats x1, Q-rows x4) close before the attention
  pools (scores x2, O-accumulators x4, transpose/output x2) open.
- bf16 for K^T/Q^T/P/V/O tiles, WO, and the transpose identity
  (transposes run 1.0 cyc/row vs 1.5 for f32r).
- A dummy Sqrt primes the sqrt-capable activation table before the
  first Square so the scalar engine loads its table once per phase.
"""

import sys

import numpy as np

for _p in ("/opt/trn_rl_repo",):
    if _p not in sys.path:
        sys.path.insert(0, _p)

SEQ, BS, DIM = 2048, 2, 256
G, SUB = 2, 4
N_CORES = 8
LN_EPS = 1e-5
NT = SEQ // 128  # 16 seq tiles of 128
NSB = SEQ // 512  # 4 blocks of 512 (query superblocks / proj blocks)

_CACHE = {}


def _build_program():
    from contextlib import ExitStack

    import concourse.bacc as bacc
    import concourse.bass_isa as bass_isa
    import concourse.mybir as mybir
    from concourse import tile
    f32 = mybir.dt.float32
    f32r = mybir.dt.float32r
    bf16 = mybir.dt.bfloat16
    AF = mybir.ActivationFunctionType
    OP = mybir.AluOpType

    nc = bacc.Bacc("TRN2", target_bir_lowering=False, debug=False)

    xt_d = nc.dram_tensor("xt", [128, 2, SEQ], bf16, kind="ExternalInput").ap()
    hdr_d = nc.dram_tensor("hdr", [128, 2560], bf16, kind="ExternalInput").ap()
    bkc_d = nc.dram_tensor("bkc", [128, 2], f32, kind="ExternalInput").ap()
    wmk_d = nc.dram_tensor("wmk", [128, 4], bf16, kind="ExternalInput").ap()
    bkm_d = nc.dram_tensor("bkm", [128, 1], f32, kind="ExternalInput").ap()
    bqc_d = nc.dram_tensor("bqc", [128, 4], f32, kind="ExternalInput").ap()
    wmq_d = nc.dram_tensor("wmq", [128, 2, 2], bf16, kind="ExternalInput").ap()
    bqm_d = nc.dram_tensor("bqm", [1, 2], f32, kind="ExternalInput").ap()
    wv_d = nc.dram_tensor("wv", [128, 2, 256], bf16, kind="ExternalInput").ap()
    wo_d = nc.dram_tensor("wo", [128, 4, 256], bf16, kind="ExternalInput").ap()
    id_d = nc.dram_tensor("ident", [128, 128], bf16, kind="ExternalInput").ap()
    cm_d = nc.dram_tensor("cmask", [128, 128], bf16, kind="ExternalInput").ap()
    vo_d = nc.dram_tensor("vpones", [128, NT, 2], bf16, kind="ExternalInput").ap()
    out_d = nc.dram_tensor("out_partial", [SEQ, DIM], bf16, kind="ExternalOutput").ap()

    r = lambda ap: ap.bitcast(f32r)

    with tile.TileContext(nc) as tc, ExitStack() as ctx:
        const = ctx.enter_context(tc.tile_pool(name="const", bufs=1))

        xt_sb = const.tile([128, 2, SEQ], bf16)
        hdr_sb = const.tile([128, 2560], bf16)
        bkc_sb = const.tile([128, 2], f32)
        wmk_sb = const.tile([128, 4], bf16)
        bkm_sb = const.tile([128, 1], f32)
        bqc_sb = const.tile([128, 4], f32)
        wmq_sb = const.tile([128, 2, 2], bf16)
        bqm_sb = const.tile([1, 2], f32)
        wv_sb = const.tile([128, 2, 256], bf16)
        wo_sb = const.tile([128, 4, 256], bf16)
        ident_sb = const.tile([128, 128], bf16)
        tmask_sb = const.tile([128, 128], bf16)
        onescol_sb = const.tile([128, 2], f32)
        epsk_sb = const.tile([128, 1], f32)
        epsq_sb = const.tile([1, 1], f32)

        # persistent SBUF activations
        ktb = [
            [const.tile([128, 512], bf16, name=f"ktb{f}_{b}") for b in range(NSB)]
            for f in range(2)
        ]
        qtn = [
            [const.tile([128, 512], bf16, name=f"qtn{fc}_{b}") for b in range(NSB)]
            for fc in range(4)
        ]
        vp_t = [const.tile([128, 258], bf16, name=f"vpt{t}") for t in range(NT)]
        rk16b = [const.tile([128, 8], f32, name=f"rk16b{b}") for b in range(NSB)]
        otb = [
            [const.tile([128, 512], bf16, name=f"otb{c}_{s}") for s in range(NSB)]
            for c in range(4)
        ]

        nc.gpsimd.memset(onescol_sb[:], 1.0)
        nc.gpsimd.memset(epsk_sb[:], 256.0 * LN_EPS)
        nc.gpsimd.memset(epsq_sb[:], LN_EPS)
        # prime the sqrt-capable activation table before any Square lands
        warm_sb = const.tile([1, 1], f32)
        nc.scalar.activation(warm_sb[:], epsq_sb[:], AF.Sqrt)
        nc.sync.dma_start(hdr_sb[:, 0:1024], hdr_d[:, 0:1024])
        nc.sync.dma_start(hdr_sb[:, 1024:1536], hdr_d[:, 1024:1536])
        nc.sync.dma_start(hdr_sb[:, 1536:2560], hdr_d[:, 1536:2560])
        nc.sync.dma_start(bkc_sb[:], bkc_d[:])
        nc.sync.dma_start(wmk_sb[:], wmk_d[:])
        nc.sync.dma_start(bkm_sb[:], bkm_d[:])
        nc.sync.dma_start(bqc_sb[:], bqc_d[:])
        nc.sync.dma_start(wmq_sb[:], wmq_d[:])
        nc.sync.dma_start(bqm_sb[:], bqm_d[:])
        nc.sync.dma_start(wv_sb[:], wv_d[:])
        nc.sync.dma_start(xt_sb[:, :, 512:1024], xt_d[:, :, 512:1024])
        nc.sync.dma_start(xt_sb[:, :, 1024:1536], xt_d[:, :, 1024:1536])
        nc.sync.dma_start(xt_sb[:, :, 1536:2048], xt_d[:, :, 1536:2048])
        nc.sync.dma_start(ident_sb[:], id_d[:])
        nc.sync.dma_start(tmask_sb[:], cm_d[:])
        for t in range(NT):
            nc.sync.dma_start(vp_t[t][:, 256:258], vo_d[:, t, :])
        nc.sync.dma_start(wo_sb[:], wo_d[:])


        def wk_v(c, f):
            return hdr_sb[:, c * 256 + f * 128 : c * 256 + (f + 1) * 128]

        def wq_v(c, lo, hi):
            return hdr_sb[:, 1536 + c * 512 + lo : 1536 + c * 512 + hi]

        def xt_v(c, lo, hi):
            if hi <= 512:
                base = 512 + c * 512
                return hdr_sb[:, base + lo : base + hi]
            return xt_sb[:, c, lo:hi]

        wrk = ctx.enter_context(tc.tile_pool(name="wrk", bufs=3))
        ppool = ctx.enter_context(tc.tile_pool(name="ppool", bufs=6))
        opool = ctx.enter_context(tc.tile_pool(name="opool", bufs=6))

        # ======== projection phase (pools scoped; close before attention) ====
        with tc.tile_pool(name="projp", bufs=3, space="PSUM") as projp, \
             tc.tile_pool(name="tinyp", bufs=1, space="PSUM") as tinyp, \
             tc.tile_pool(name="rowp", bufs=1, space="PSUM") as rowp:
            for B in range(NSB):
                sl = slice(B * 512, (B + 1) * 512)
                # --- K^T chunks: biased SBUF copy + biased square (from PSUM)
                ktsq = []
                for f in range(2):
                    kps = projp.tile([128, 512], f32, tag="proj", name=f"kps{f}_{B}")
                    for c in range(2):
                        nc.tensor.matmul(
                            kps[:],
                            lhsT=wk_v(c, f),
                            rhs=xt_v(c, B * 512, (B + 1) * 512),
                            start=(c == 0),
                            stop=(c == 1),
                        )
                    ksq = wrk.tile([128, 512], f32r, tag=f"ksq{f}", bufs=2,
                                   name=f"ksq{f}_{B}")
                    nc.scalar.activation(
                        ksq[:], kps[:], AF.Square, bias=bkc_sb[:, f : f + 1]
                    )
                    nc.vector.tensor_scalar_add(
                        ktb[f][B][:], kps[:], scalar1=bkc_sb[:, f : f + 1]
                    )
                    ktsq.append(ksq)
                # --- Q^T chunks: raw in PSUM until normalize; biased square
                qps_l, qtsq = [], []
                for fc in range(4):
                    qps = projp.tile([128, 512], f32, tag="proj", name=f"qps{fc}_{B}")
                    for c in range(2):
                        nc.tensor.matmul(
                            qps[:],
                            lhsT=wq_v(c, fc * 128, (fc + 1) * 128),
                            rhs=xt_v(c, B * 512, (B + 1) * 512),
                            start=(c == 0),
                            stop=(c == 1),
                        )
                    qsq = wrk.tile([128, 512], f32r, tag=f"qsq{fc}", bufs=2,
                                   name=f"qsq{fc}_{B}")
                    nc.scalar.activation(
                        qsq[:], qps[:], AF.Square, bias=bqc_sb[:, fc : fc + 1]
                    )
                    qps_l.append(qps)
                    qtsq.append(qsq)
                # --- V tiles ---
                for t in range(4 * B, 4 * B + 4):
                    vps = projp.tile([128, 256], f32, tag="proj", name=f"vps{t}")
                    for c in range(2):
                        nc.tensor.matmul(
                            vps[:],
                            lhsT=xt_v(c, t * 128, (t + 1) * 128),
                            rhs=wv_sb[:, c, :],
                            start=(c == 0),
                            stop=(c == 1),
                        )
                    nc.scalar.copy(vp_t[t][:, 0:256], vps[:])
                # --- K per-key stats: mu (cols 0:4) and sumsq (cols 4:8) ---
                tiny = tinyp.tile([128, 16], f32, tag="tiny", name=f"tiny{B}")
                for i, t in enumerate(range(4 * B, 4 * B + 4)):
                    for c in range(2):
                        nc.tensor.matmul(
                            tiny[:, 2 * i : 2 * i + 2],
                            lhsT=xt_v(c, t * 128, (t + 1) * 128),
                            rhs=wmk_sb[:, 2 * c : 2 * c + 2],
                            start=(c == 0),
                            stop=(c == 1),
                        )
                    for f in range(2):
                        nc.tensor.matmul(
                            tiny[:, 8 + 2 * i : 10 + 2 * i],
                            lhsT=r(ktsq[f][:, i * 128 : (i + 1) * 128]),
                            rhs=r(onescol_sb[:, 0:2]),
                            start=(f == 0),
                            stop=(f == 1),
                        )
                # --- Q row stats, one bank per head: mu at partition 0,
                # sumsq at partition 32 (matmul base must be 0/32/64)
                mur_h = [
                    rowp.tile([1, 512], f32, tag=f"mur{h}", name=f"mur{h}_{B}")[:]
                    for h in range(2)
                ]
                for h in range(2):
                    for c in range(2):
                        nc.tensor.matmul(
                            mur_h[h],
                            lhsT=wmq_sb[:, c, h : h + 1],
                            rhs=xt_v(c, B * 512, (B + 1) * 512),
                            start=(c == 0),
                            stop=(c == 1),
                        )
                sqr_h = [
                    rowp.tile([1, 512], f32, tag=f"sqr{h}", name=f"sqr{h}_{B}")[:]
                    for h in range(2)
                ]
                for h in range(2):
                    for c in range(2):
                        nc.tensor.matmul(
                            sqr_h[h],
                            lhsT=r(onescol_sb[:, 0:1]),
                            rhs=r(qtsq[2 * h + c][:]),
                            start=(c == 0),
                            stop=(c == 1),
                        )
                # --- K stats -> rk16 (batched over the 4 seq tiles) ---
                mu2 = wrk.tile([128, 8], f32, tag="mu2", name=f"mu2_{B}")
                nc.scalar.activation(mu2[:], tiny[:, 0:8], AF.Square, bias=bkm_sb[:])
                v256 = wrk.tile([128, 8], f32, tag="v256", name=f"v256_{B}")
                nc.vector.scalar_tensor_tensor(
                    v256[:], mu2[:], -256.0, tiny[:, 8:16],
                    op0=OP.mult, op1=OP.add,
                )
                std16 = wrk.tile([128, 8], f32, tag="std16", name=f"std16_{B}")
                nc.scalar.activation(std16[:], v256[:], AF.Sqrt, bias=epsk_sb[:])
                nc.vector.reciprocal(rk16b[B][:], std16[:])
                # --- Q row stats -> (mu_biased, 1/std) rows + broadcasts ---
                mub = [None, None]
                rqb = [None, None]
                for h in range(2):
                    murow = wrk.tile([1, 512], f32, tag=f"murow{h}", bufs=1,
                                     name=f"murow{h}_{B}")
                    nc.vector.tensor_scalar_add(
                        murow[:], mur_h[h],
                        scalar1=bqm_sb[0:1, h : h + 1],
                    )
                    mu2r = wrk.tile([1, 512], f32, tag=f"mu2r{h}", bufs=1,
                                    name=f"mu2r{h}_{B}")
                    nc.scalar.activation(mu2r[:], murow[:], AF.Square)
                    v256r = wrk.tile([1, 512], f32, tag=f"v256r{h}", bufs=1,
                                     name=f"v256r{h}_{B}")
                    nc.vector.scalar_tensor_tensor(
                        v256r[:], mu2r[:], -256.0, sqr_h[h],
                        op0=OP.mult, op1=OP.add,
                    )
                    stdr = wrk.tile([1, 512], f32, tag=f"stdr{h}", bufs=1,
                                    name=f"stdr{h}_{B}")
                    nc.scalar.activation(
                        stdr[:], v256r[:], AF.Sqrt, bias=epsq_sb[:],
                        scale=1.0 / 256.0,
                    )
                    rqrow = wrk.tile([1, 512], f32, tag=f"rqrow{h}", bufs=1,
                                     name=f"rqrow{h}_{B}")
                    nc.vector.reciprocal(rqrow[:], stdr[:])
                    mub[h] = wrk.tile([128, 512], f32, tag=f"mub{h}", bufs=1,
                                      name=f"mub{h}_{B}")
                    nc.gpsimd.partition_broadcast(mub[h][:], murow[:])
                    rqb[h] = wrk.tile([128, 512], f32, tag=f"rqb{h}", bufs=1,
                                      name=f"rqb{h}_{B}")
                    nc.gpsimd.partition_broadcast(rqb[h][:], rqrow[:])
                # --- normalize Q: ((raw + bias) - mu) * (1/std) -> bf16 ---
                for fc in range(4):
                    h = fc // 2
                    qtmp = wrk.tile([128, 512], f32, tag=f"qtmp{fc % 2}",
                                    name=f"qtmp{fc}_{B}")
                    nc.vector.scalar_tensor_tensor(
                        qtmp[:], qps_l[fc][:], bqc_sb[:, fc : fc + 1], mub[h][:],
                        op0=OP.add, op1=OP.subtract,
                    )
                    nc.gpsimd.tensor_mul(qtn[fc][B][:], qtmp[:], rqb[h][:])

        # ======== attention phase ========
        psA = ctx.enter_context(tc.tile_pool(name="psA", bufs=2, space="PSUM"))
        psB = ctx.enter_context(tc.tile_pool(name="psB", bufs=1, space="PSUM"))
        psT = ctx.enter_context(tc.tile_pool(name="psT", bufs=2, space="PSUM"))

        def attn_superblock(h, s, inject=None):
            n_k = 4 * (s + 1)
            oacc = [
                psB.tile([128, 258], f32, tag=f"oacc{j}", name=f"oacc{h}_{s}_{j}")
                for j in range(4)
            ]
            p_tiles = [None] * n_k

            def issue_scores(kt):
                d = kt - 4 * s  # >= 0 on the diagonal region
                qoff = 0 if d <= 0 else d * 128  # bf16 moving: 1 cyc/row anyway
                st = psA.tile([128, 512], f32, tag="mm512", name=f"st{h}_{s}_{kt}")
                for c in range(2):
                    nc.tensor.matmul(
                        st[:, qoff:512],
                        lhsT=ktb[c][kt // 4][:, (kt % 4) * 128 : (kt % 4 + 1) * 128],
                        rhs=qtn[h * 2 + c][s][:, qoff:512],
                        start=(c == 0),
                        stop=(c == 1),
                    )
                p = ppool.tile([128, 512], bf16, tag="p", name=f"p{h}_{s}_{kt}")
                nc.scalar.activation(
                    p[:, qoff:512], st[:, qoff:512], AF.Exp,
                    scale=rk16b[kt // 4][:, 2 * (kt % 4) : 2 * (kt % 4) + 1],
                )
                if d >= 0:
                    nc.vector.tensor_mul(
                        p[:, d * 128 : (d + 1) * 128],
                        p[:, d * 128 : (d + 1) * 128],
                        tmask_sb[:],
                    )
                p_tiles[kt] = p

            def issue_pv(kt):
                d = kt - 4 * s
                p = p_tiles[kt]
                for j in range(max(d, 0), 4):
                    nc.tensor.matmul(
                        oacc[j][:],
                        lhsT=p[:, j * 128 : (j + 1) * 128],
                        rhs=vp_t[kt][:],
                        start=(kt == 0),
                        stop=(kt == 4 * s + j),
                    )

            for kk in range(min(3, n_k)):
                issue_scores(kk)
            if inject is not None:
                inject()
            osb_list = [None] * 4

            def finish_j(j):
                rc = wrk.tile([128, 1], f32, tag="rc", name=f"rc{h}_{s}_{j}")
                nc.vector.reciprocal(rc[:], oacc[j][:, 256:257])
                osb = opool.tile([128, 256], bf16, tag="osb", name=f"osb{h}_{s}_{j}")
                nc.vector.tensor_scalar_mul(osb[:], oacc[j][:, 0:256], rc[:])
                osb_list[j] = osb

            for kt in range(n_k):
                issue_pv(kt)
                if kt + 3 < n_k:
                    issue_scores(kt + 3)
                if kt >= 4 * s:
                    finish_j(kt - 4 * s)
            return osb_list

        def o_trans(h, s, osb_list):
            """transpose the 4 normalized output tiles of (h, s) into otb"""
            for c in range(2):
                big = psT.tile([128, 512], bf16, tag="tp", name=f"obig{h}_{s}_{c}")
                for j in range(4):
                    nc.tensor.matmul(
                        big[:, j * 128 : (j + 1) * 128],
                        lhsT=osb_list[j][:, c * 128 : (c + 1) * 128],
                        rhs=ident_sb[:],
                        is_transpose=True,
                    )
                nc.vector.tensor_copy(otb[h * 2 + c][s][:], big[:])

        def o_proj(t):
            s, j = t // 4, t % 4
            ops = psT.tile([128, 256], f32, tag="tp", name=f"ops{t}")
            for c in range(4):
                nc.tensor.matmul(
                    ops[:],
                    lhsT=otb[c][s][:, j * 128 : (j + 1) * 128],
                    rhs=wo_sb[:, c, :],
                    start=(c == 0),
                    stop=(c == 3),
                )
            outsb = opool.tile([128, 256], bf16, tag="outsb", name=f"outsb{t}")
            nc.scalar.copy(outsb[:], ops[:])
            nc.sync.dma_start(out_d[t * 128 : (t + 1) * 128, :], outsb[:])

        # pipeline: h0(s) transposes + h1(s-1) transposes + o_proj(s-1) all
        # run inside later instruction streams so their DVE inputs are ready.
        osb_mem = {}
        for s in range(NSB):

            def inj_h0(ss=s):
                if ss > 0:
                    o_trans(1, ss - 1, osb_mem[(1, ss - 1)])

            def inj_h1(ss=s):
                if ss > 0:
                    for t in range(4 * (ss - 1), 4 * ss):
                        o_proj(t)
                o_trans(0, ss, osb_mem[(0, ss)])

            osb_mem[(0, s)] = attn_superblock(0, s, inj_h0)
            osb_mem[(1, s)] = attn_superblock(1, s, inj_h1)

        # drain tail, j-granular: transpose/copy/project/store per query tile
        s_last = NSB - 1
        osb_l = osb_mem[(1, s_last)]

        def tail_trans(j):
            jb = psT.tile([128, 256], bf16, tag="tp", name=f"jb{j}")
            for c in range(2):
                nc.tensor.matmul(
                    jb[:, c * 128 : (c + 1) * 128],
                    lhsT=osb_l[j][:, c * 128 : (c + 1) * 128],
                    rhs=ident_sb[:],
                    is_transpose=True,
                )
            for c in range(2):
                nc.vector.tensor_copy(
                    otb[2 + c][s_last][:, j * 128 : (j + 1) * 128],
                    jb[:, c * 128 : (c + 1) * 128],
                )

        for j in range(4):
            tail_trans(j)
            o_proj(4 * s_last + j)

    nc.finalize()
    return nc


def _chunk2(a):
    """[256, F] -> [128, 2, F] (feature chunks on the free axis)."""
    f = a.shape[1]
    return np.ascontiguousarray(a.reshape(2, 128, f).transpose(1, 0, 2))


def _prep_core_inputs(c, x, WK_w, WK_b, WV_w, WV_b, WQ_w, WQ_b, WO_w):
    import ml_dtypes

    bf16 = ml_dtypes.bfloat16
    b, g, j = c // 4, (c // 2) % 2, c % 2
    f32 = np.float32

    xT = np.ascontiguousarray(x[:, b, :].T.astype(f32))  # [256, 2048]
    xt = _chunk2(xT).astype(bf16)

    wk_s = WK_w[:, g * 256 : (g + 1) * 256].astype(f32)  # [256 in, 256 out]
    bk_s = WK_b[g * 256 : (g + 1) * 256].astype(f32)
    wv_s = np.ascontiguousarray(WV_w[:, g * 256 : (g + 1) * 256].astype(f32))

    col = (g * SUB + 2 * j) * 256
    wq_s = WQ_w[:, col : col + 512].astype(f32)  # both heads [256 in, 512 out]
    bq_s = WQ_b[col : col + 512].astype(f32)

    row = (g * SUB + 2 * j) * 256
    wo_s = WO_w[row : row + 512, :].astype(f32)  # [512, 256]
    wo = np.ascontiguousarray(wo_s.reshape(4, 128, 256).transpose(1, 0, 2))

    pp, ff = np.meshgrid(np.arange(128), np.arange(128), indexing="ij")
    hdr = np.concatenate(
        [
            np.ascontiguousarray(_chunk2(wk_s).reshape(128, 512)).astype(bf16),
            xt[:, 0, 0:512],
            xt[:, 1, 0:512],
            np.ascontiguousarray(_chunk2(wq_s).reshape(128, 1024)).astype(bf16),
        ],
        axis=1,
    )
    return {
        "xt": xt,
        "hdr": np.ascontiguousarray(hdr),
        "bkc": np.ascontiguousarray(bk_s.reshape(2, 128).T),
        "wmk": np.ascontiguousarray(
            np.repeat(wk_s.mean(axis=1, keepdims=True).reshape(2, 128).T, 2, axis=1)
        ).astype(bf16),
        "bkm": np.full((128, 1), bk_s.mean(), dtype=f32),
        "bqc": np.ascontiguousarray(bq_s.reshape(4, 128).T),
        "wmq": np.ascontiguousarray(
            wq_s.reshape(256, 2, 256).mean(axis=2).reshape(2, 128, 2).transpose(1, 0, 2)
        ).astype(bf16),
        "bqm": np.array(
            [[bq_s[0:256].mean(), bq_s[256:512].mean()]], dtype=f32
        ),
        "wv": _chunk2(wv_s).astype(bf16),
        "wo": wo.astype(bf16),
        "ident": np.eye(128, dtype=bf16),
        "cmask": (pp <= ff).astype(bf16),  # keep k<=q on the diagonal tile
        "vpones": np.concatenate(
            [np.ones((128, NT, 1), dtype=bf16), np.zeros((128, NT, 1), dtype=bf16)],
            axis=2,
        ),
    }


def kernel(x, WK_w, WK_b, WV_w, WV_b, WQ_w, WQ_b, WO_w, WO_b, ln_g, ln_b, **kwargs):
    x = np.asarray(x)
    WK_w, WK_b = np.asarray(WK_w), np.asarray(WK_b)
    WV_w, WV_b = np.asarray(WV_w), np.asarray(WV_b)
    WQ_w, WQ_b = np.asarray(WQ_w), np.asarray(WQ_b)
    WO_w, WO_b = np.asarray(WO_w), np.asarray(WO_b)
    ln_g, ln_b = np.asarray(ln_g), np.asarray(ln_b)

    if not np.allclose(ln_b, 0.0):
        raise NotImplementedError("nonzero ln_b not supported by this kernel")
    if not np.allclose(ln_g, 1.0):
        raise NotImplementedError("non-unit ln_g not supported by this kernel")

    if "nc" not in _CACHE:
        _CACHE["nc"] = _build_program()
    nc = _CACHE["nc"]

    in_maps = [
        _prep_core_inputs(c, x, WK_w, WK_b, WV_w, WV_b, WQ_w, WQ_b, WO_w)
        for c in range(N_CORES)
    ]

    from concourse.bass_utils import run_bass_kernel_spmd

    res = run_bass_kernel_spmd(nc, in_maps, list(range(N_CORES)))
    results = res.results

    out = np.zeros((SEQ, BS, DIM), dtype=np.float32)
    for c in range(N_CORES):
        out[:, c // 4, :] += np.asarray(results[c]["out_partial"], dtype=np.float32)

    # fold: WO_b plus the V-bias contribution of every head
    const_bias = WO_b.astype(np.float64).copy()
    for g in range(G):
        bv = WV_b[g * 256 : (g + 1) * 256].astype(np.float64)
        for sh in range(SUB):
            row = (g * SUB + sh) * 256
            const_bias += bv @ WO_w[row : row + 256, :].astype(np.float64)
    out += const_bias.astype(np.float32)[None, None, :]
    return out
